# revision 1
# baseline (speedup 1.0000x reference)
"""CapsuleLayer dynamic-routing kernel for Trainium2 (Bass/Tile), SPMD over 8 cores.

Math (per batch sample, from the reference):
    u_hat[j, (i,k)] = sum_k' x[j, k'] * W[k', (i,k)]        j=1024, k'=256, (i,k)=16x32=512
    b_0 = 0
    for t in 0..3:
        c = softmax_i(b)                                    [16, 1024]
        s[i, k] = sum_j c[i, j] * u_hat[j, (i,k)]
        v = s / sqrt(sum_k s^2 + eps)                       [16, 32]
        if t < 3: b[i, j] = sum_k v[i, k] * u_hat[j, (i,k)]
    return v

Sharding: data-parallel over batch (128 -> 16 per core), W replicated.

Per-core layout strategy (all matmul inputs fp16, PSUM fp32):
  - x arrives fp16; xT comes straight off the DMA xbar transpose from DRAM
  - u_hat  [j-part, (i,k)]  via matmul lhsT=xT-chunks rhs=W (fp16, fp32 PSUM)
  - u_hatT [(i,k)-part, j]  via PE transposes of u_hat
  - routing processes 4 samples per group, packed 32-per-sample in PSUM
    partitions with col-group tile_position for concurrent PE strips; each
    bank runs ONE accumulation group (start=True only on the very first
    matmul) so per-element has_written gives first-touch-overwrite.
  - s-matrix masked to its block diagonal, giving both the squash norms and
    (via PE transpose) the block-diagonal lhsT for the b-update.
  - softmax runs in the bT layout [j-part, (sample, i)]; 1/Z folded in with a
    free-dim-broadcast multiply; 1/||s|| folded into the b copy as a
    per-partition activation scale; rsqrt via magic-constant + Newton on the
    DVE so every ScalarE function stays in one activation-table set.
"""

import functools

import numpy as np

import concourse.bass as bass
import concourse.mybir as mybir
import concourse.tile as tile
from concourse import bacc
from concourse.bass_utils import run_bass_kernel_spmd

F32 = mybir.dt.float32
I32 = mybir.dt.int32
F16 = mybir.dt.float16
AF = mybir.ActivationFunctionType
ALU = mybir.AluOpType
AX = mybir.AxisListType
ts = bass.ts

NCORES = 8
BFULL = 128
BSH = BFULL // NCORES  # 16 samples per core
NJ, NK, ND = 1024, 256, 512  # j, k', (i,k)
NI, DK = 16, 32
JT, KT, IKT = NJ // 128, NK // 128, ND // 128  # 8, 2, 4
GS = 4  # samples per routing group (packed in PSUM partitions at 32-stride)
NG = BSH // GS  # 4
ROUTINGS = 4
EPS = 1e-7
P = 128


def _build_body(nc, tc, x_ap, w_ap, ident_ap, sel_ap, mask_ap, out_ap, ctx):
    consts = ctx.enter_context(tc.tile_pool(name="consts", bufs=1))
    xT = ctx.enter_context(tc.tile_pool(name="xT", bufs=4))
    uhp = ctx.enter_context(tc.tile_pool(name="uh", bufs=2 * GS))
    uhTp = ctx.enter_context(tc.tile_pool(name="uhT", bufs=2 * GS))
    rt = ctx.enter_context(tc.tile_pool(name="rt", bufs=3))
    sm = ctx.enter_context(tc.tile_pool(name="sm", bufs=3))
    psum = ctx.enter_context(tc.tile_pool(name="psum", bufs=2, space="PSUM"))

    # ---- constants ----
    ident = consts.tile([P, P], F16)
    nc.sync.dma_start(ident[:], ident_ap)
    sel = consts.tile([P, DK], F16)
    nc.sync.dma_start(sel[:], sel_ap)
    mask = consts.tile([P, ND], F16)
    nc.sync.dma_start(mask[:], mask_ap)
    w32 = consts.tile([P, KT, ND], F32)
    nc.sync.dma_start(w32[:], w_ap.rearrange("(t p) d -> p t d", p=P))
    wf = consts.tile([P, KT, ND], F16)
    nc.scalar.copy(wf.rearrange("p t d -> p (t d)"), w32.rearrange("p t d -> p (t d)"))

    epsb = consts.tile([P, 1], F32)
    nc.gpsimd.memset(epsb[:], EPS)


    uh_tiles = [None] * BSH
    uhT_tiles = [None] * BSH

    def stage1(s):
        """x[s] -> xT fp16 -> u_hat [j,(ik)] fp16 -> u_hatT [(ik),j] fp16."""
        # xbar transpose straight from DRAM: out[p, kt, q] = x[q, 128*kt + p]
        xTt = xT.tile([P, KT, NJ], F16, name="xT")
        nc.sync.dma_start_transpose(xTt[:], x_ap[s])

        uh = uhp.tile([P, JT, ND], F16, name="uh")
        for jt in range(JT):
            pu = psum.tile([P, ND], F32, name="pu", tag="pu", bufs=3)
            for kt in range(KT):
                nc.tensor.matmul(
                    pu[:],
                    lhsT=xTt[:, kt, ts(jt, P)],
                    rhs=wf[:, kt, :],
                    start=(kt == 0),
                    stop=(kt == KT - 1),
                )
            if jt in (0, 3, 6):
                nc.vector.tensor_copy(uh[:, jt, :], pu[:])
            else:
                nc.scalar.copy(uh[:, jt, :], pu[:])

        uhT = uhTp.tile([P, IKT, NJ], F16, name="uhT")
        for dt in range(IKT):
            pt2 = psum.tile([P, NJ], F16, name="pt2", tag="pf16", bufs=2)
            for jt in range(JT):
                nc.tensor.transpose(pt2[:, ts(jt, P)], uh[:, jt, ts(dt, P)], ident[:])
            nc.vector.tensor_copy(uhT[:, dt, :], pt2[:])
        uh_tiles[s] = uh
        uhT_tiles[s] = uhT

    def routing(g):
        samples = [g * GS + i for i in range(GS)]
        # two cT tiles per group, alternating across iterations; cols 16..31
        # stay zero so M=32 col-strip matmuls write the whole PSUM bank
        ct_tiles = [
            sm.tile([P, JT, GS, 32], F16, name="ct", tag="ct", bufs=4)
            for _ in range(2)
        ]
        for tt in ct_tiles:
            nc.gpsimd.memset(tt[:], 0.0)
        nc.gpsimd.memset(ct_tiles[0][:, :, :, 0:NI], 1.0 / NI)
        for t in range(ROUTINGS):
            ct = ct_tiles[t % 2]

            # s-einsum: 4 samples concurrent in one PSUM bank via col groups.
            # One accumulation group for the whole bank: start=True only on the
            # very first matmul (clears has_written bank-wide); later matmuls
            # first-touch-overwrite their strip, then accumulate. The memset
            # keeps CoreSim (whose pending-zero tracking is partition-blind)
            # numerically in agreement.
            ps_s = psum.tile([P, ND], F32, name="ps_s", tag="prt", bufs=3)
            nc.vector.memset(ps_s[:], 0.0)
            for jt in range(JT):
                for a in range(GS):
                    nc.tensor.matmul(
                        ps_s[ts(a, 32), :],
                        lhsT=ct[:, jt, a, :],
                        rhs=uh_tiles[samples[a]][:, jt, :],
                        start=(jt == 0 and a == 0),
                        stop=(jt == JT - 1 and a == GS - 1),
                        tile_position=(0, 32 * a),
                        skip_group_check=True,
                    )

            # mask to block diagonal; norms; rinv = (n2+eps)^-0.5
            masked = rt.tile([P, ND], F16, name="masked")
            nc.vector.tensor_tensor(masked[:], ps_s[:], mask[:], op=ALU.mult)
            sq = rt.tile([P, ND], F16, name="sq")
            n2 = rt.tile([P, 1], F32, name="n2")
            nc.scalar.activation(sq[:], masked[:], AF.Square, accum_out=n2[:])
            # rinv = (n2+eps)^-0.5 on DVE: magic-constant guess + 3 Newton
            # steps (keeps ScalarE funcs inside one activation-table set)
            xe = rt.tile([P, 1], F32, name="xe")
            nc.vector.tensor_scalar(xe[:], n2[:], EPS, None, op0=ALU.add)
            xh = rt.tile([P, 1], F32, name="xh")
            nc.vector.tensor_scalar(xh[:], xe[:], 0.5, None, op0=ALU.mult)
            yt = rt.tile([P, 1], F32, name="yt")
            nc.vector.tensor_scalar(
                yt.bitcast(I32)[:], xe.bitcast(I32)[:], 1, None,
                op0=ALU.logical_shift_right,
            )
            nc.vector.tensor_scalar(
                yt.bitcast(I32)[:], yt.bitcast(I32)[:], 0x5F3759E0, None,
                op0=ALU.subtract,
            )
            nc.vector.tensor_scalar(
                yt.bitcast(I32)[:], yt.bitcast(I32)[:], -1, None,
                op0=ALU.bitwise_xor,
            )
            y2 = rt.tile([P, 1], F32, name="y2")
            for _ in range(2):
                nc.vector.tensor_tensor(y2[:], yt[:], yt[:], op=ALU.mult)
                nc.vector.tensor_tensor(y2[:], y2[:], xh[:], op=ALU.mult)
                nc.vector.tensor_scalar(y2[:], y2[:], -1.0, 1.5, op0=ALU.mult, op1=ALU.add)
                nc.vector.tensor_tensor(yt[:], yt[:], y2[:], op=ALU.mult)
            rinv = yt

            # block-diagonal V (unnormalized): PE transpose of masked
            pv = psum.tile([P, IKT * P], F16, name="pv", tag="pf16", bufs=2)
            for c in range(IKT):
                nc.tensor.transpose(pv[:, ts(c, P)], masked[:, ts(c, P)], ident[:])
            vblk = rt.tile([P, IKT, P], F16, name="vblk")
            nc.scalar.copy(vblk.rearrange("p t c -> p (t c)"), pv[:])

            if t == ROUTINGS - 1:
                # final squash output: diag-extract via matmul with Sel, scale
                ps_v = psum.tile([P, DK], F32, name="ps_v", tag="prt", bufs=3)
                for kt in range(IKT):
                    nc.tensor.matmul(
                        ps_v[:],
                        lhsT=vblk[:, kt, :],
                        rhs=sel[:],
                        start=(kt == 0),
                        stop=(kt == IKT - 1),
                    )
                vout = rt.tile([P, DK], F32, name="vout")
                nc.scalar.activation(vout[:], ps_v[:], AF.Copy, scale=rinv[:])
                for a in range(GS):
                    nc.gpsimd.dma_start(
                        out_ap[samples[a]], vout[32 * a : 32 * a + NI, :]
                    )
                continue

            # b-update: b[i,j] = sum_k v u_hatT ; exp(rinv*b) fused into the
            # PSUM evacuation (softmax needs only exp(b*rinv))
            bsc = rt.tile([P, 2, ND], F16, name="bsc")
            for jc in range(2):
                ps_b = psum.tile([P, ND], F32, name="ps_b", tag="prt", bufs=3)
                nc.scalar.activation(
                    ps_b[:], epsb.broadcast_to([P, ND]), AF.Copy, scale=0.0
                )
                for kt in range(IKT):
                    for a in range(GS):
                        nc.tensor.matmul(
                            ps_b[ts(a, 32), :],
                            lhsT=vblk[:, kt, ts(a, 32)],
                            rhs=uhT_tiles[samples[a]][:, kt, ts(jc, ND)],
                            start=(kt == 0 and a == 0),
                            stop=(kt == IKT - 1 and a == GS - 1),
                            tile_position=(0, 32 * a),
                            skip_group_check=True,
                        )
                nc.scalar.activation(bsc[:, jc, :], ps_b[:], AF.Exp, scale=rinv[:])

            # transpose to bT [j-part, (sample,i)] and softmax over i
            pbt = psum.tile([P, JT, P], F16, name="pbt", tag="pf16", bufs=2)
            for jt in range(JT):
                nc.tensor.transpose(
                    pbt[:, jt, :], bsc[:, jt // 4, ts(jt % 4, P)], ident[:]
                )
            expT = pbt.rearrange("p t (s c) -> p t s c", c=32)[:, :, :, 0:NI]
            zsum = sm.tile([P, JT, GS], F32, name="zsum")
            nc.vector.tensor_reduce(zsum[:], expT, axis=AX.X, op=ALU.add)
            rz = sm.tile([P, JT, GS], F32, name="rz")
            nc.vector.reciprocal(rz[:], zsum[:])
            ct_next = ct_tiles[(t + 1) % 2]
            nc.vector.tensor_tensor(
                ct_next[:, :, :, 0:NI],
                expT,
                rz.unsqueeze(3).broadcast_to([P, JT, GS, NI]),
                op=ALU.mult,
            )

    for g in range(NG):
        for s in range(g * GS, (g + 1) * GS):
            stage1(s)
        routing(g)


def _np_consts():
    ident = np.eye(P, dtype=ml_dtypes_f16())
    sel = np.tile(np.eye(DK, dtype=ml_dtypes_f16()), (IKT, 1))
    mask = np.zeros((P, ND), dtype=ml_dtypes_f16())
    for a in range(GS):
        for i in range(NI):
            mask[32 * a + i, DK * i : DK * (i + 1)] = 1.0
    return ident, sel, mask


def ml_dtypes_f16():
    return np.float16


@functools.cache
def _build_nc():
    from contextlib import ExitStack

    nc = bacc.Bacc(
        "TRN2",
        target_bir_lowering=False,
        debug=False,
        num_devices=NCORES,
    )
    x_t = nc.dram_tensor("x", [BSH, NJ, NK], F16, kind="ExternalInput")
    w_t = nc.dram_tensor("w", [NK, ND], F32, kind="ExternalInput")
    ident_t = nc.dram_tensor("ident", [P, P], F16, kind="ExternalInput")
    sel_t = nc.dram_tensor("sel", [P, DK], F16, kind="ExternalInput")
    mask_t = nc.dram_tensor("mask", [P, ND], F16, kind="ExternalInput")
    out_t = nc.dram_tensor("out", [BSH, NI, DK], F32, kind="ExternalOutput")

    with tile.TileContext(nc) as tc:
        with ExitStack() as ctx:
            _build_body(
                nc,
                tc,
                x_t.ap(),
                w_t.ap(),
                ident_t.ap(),
                sel_t.ap(),
                mask_t.ap(),
                out_t.ap(),
                ctx,
            )
    nc.compile()
    return nc


def _in_maps(x, W):
    x = np.ascontiguousarray(np.asarray(x, dtype=np.float32))
    w2d = np.ascontiguousarray(np.asarray(W, dtype=np.float32).reshape(NK, ND))
    ident, sel, mask = _np_consts()
    maps = []
    for c in range(NCORES):
        maps.append(
            {
                "x": np.ascontiguousarray(x[c * BSH : (c + 1) * BSH]).astype(np.float16),
                "w": w2d,
                "ident": ident,
                "sel": sel,
                "mask": mask,
            }
        )
    return maps


def run(x, W, trace=False):
    nc = _build_nc()
    res = run_bass_kernel_spmd(nc, _in_maps(x, W), list(range(NCORES)), trace=trace)
    out = np.concatenate([r["out"] for r in res.results], axis=0)
    return out.astype(np.float32), res


def kernel(x, W):
    out, _ = run(x, W, trace=False)
    return out



# revision 6
# speedup vs baseline: 1.6154x; 1.6154x over previous
"""CapsuleLayer dynamic-routing kernel for Trainium2 (Bass/Tile), SPMD over 8 cores.

Math (per batch sample, from the reference):
    u_hat[j, (i,k)] = sum_k' x[j, k'] * W[k', (i,k)]        j=1024, k'=256, (i,k)=16x32=512
    b_0 = 0
    for t in 0..3:
        c = softmax_i(b)                                    [16, 1024]
        s[i, k] = sum_j c[i, j] * u_hat[j, (i,k)]
        v = s / sqrt(sum_k s^2 + eps)                       [16, 32]
        if t < 3: b[i, j] = sum_k v[i, k] * u_hat[j, (i,k)]
    return v

KEY STRUCTURE: u_hat is never materialized. Both routing contractions factor
through W:
    s = (C x) W          (contract j against x [256 wide], then k'=256 with W)
    b = (W v) . x        (tiny w_v = W^T-blocks . v pass, then k'=256 with xT)
This removes the u_hat matmuls, all PE transposes of u_hat, and the PSUM
evacuation traffic that dominated the direct implementation.

Sharding: data-parallel over batch (128 -> 16 per core), W replicated.

Per-core schedule: 4 groups x 4 samples (col-strip packed, tile_position),
4 routing rounds emitted stage-major round-robin across groups so every
engine queue (PE / DVE / ScalarE / sync-DMA) sees work in readiness order.
All transposes (cx->cxT, masked->maskedT, b->bT) are DMA xbar transposes
(SBUF->SBUF) on the sync queue - zero PE time. xT is pre-transposed on the
host so both x layouts stream from DRAM with plain DMAs.
"""

import functools

import numpy as np

import concourse.bass as bass
import concourse.mybir as mybir
import concourse.tile as tile
from concourse import bacc
from concourse.bass_utils import run_bass_kernel_spmd

F32 = mybir.dt.float32
I32 = mybir.dt.int32
F16 = mybir.dt.float16
AF = mybir.ActivationFunctionType
ALU = mybir.AluOpType
AX = mybir.AxisListType
ts = bass.ts

NCORES = 8
BFULL = 128
BSH = BFULL // NCORES  # 16 samples per core
NJ, NK, ND = 1024, 256, 512  # j, k', (i,k)
NI, DK = 16, 32
JT, KT, IKT = NJ // 128, NK // 128, ND // 128  # 8, 2, 4
GS = 4  # samples per routing group (packed in PSUM partitions at 32-stride)
NG = BSH // GS  # 4
ROUTINGS = 4
EPS = 1e-7
P = 128


def _build_body(nc, tc, x_ap, xt_ap, w_ap, mask_ap, sel_ap, out_ap, ctx):
    consts = ctx.enter_context(tc.tile_pool(name="consts", bufs=1))
    xp = ctx.enter_context(tc.tile_pool(name="xp", bufs=NG))
    sm = ctx.enter_context(tc.tile_pool(name="sm", bufs=2 * NG))
    rt = ctx.enter_context(tc.tile_pool(name="rt", bufs=4))
    psum = ctx.enter_context(tc.tile_pool(name="psum", bufs=2, space="PSUM"))

    # ---- constants (w32 first: the wf cast gates wT + C-stage) ----
    w32 = consts.tile([P, KT, ND], F32)
    nc.sync.dma_start(w32[:], w_ap.rearrange("(t p) d -> p t d", p=P))
    mask = consts.tile([P, ND], F16)
    nc.sync.dma_start(mask[:], mask_ap)
    sel = consts.tile([P, DK], F16)
    nc.sync.dma_start(sel[:], sel_ap)
    wf = consts.tile([P, KT, ND], F16)
    nc.scalar.copy(wf.rearrange("p t d -> p (t d)"), w32.rearrange("p t d -> p (t d)"))

    # ---- per-group x in both layouts (xT pre-transposed on host) ----
    xn_g = [None] * NG  # [P(j%128), GS, JT, NK]  natural
    xt_g = [None] * NG  # [P(k'%128), GS, KT, NJ] transposed
    for g in range(NG):
        xn = xp.tile([P, GS, JT, NK], F16, name="xn")
        nc.gpsimd.dma_start(
            xn[:], x_ap[g * GS : (g + 1) * GS].rearrange("s (t p) k -> p s t k", p=P)
        )
        xt = xp.tile([P, GS, KT, NJ], F16, name="xt")
        nc.gpsimd.dma_start(
            xt[:], xt_ap[g * GS : (g + 1) * GS].rearrange("s (t p) j -> p s t j", p=P)
        )
        xn_g[g] = xn
        xt_g[g] = xt
        if g == 1:
            # wT[p(ik%128), ikt, kt, q(k'%128)] = W[128kt+q, 128ikt+p]
            wT = consts.tile([P, IKT, KT, P], F16)
            for kt in range(KT):
                nc.sync.dma_start_transpose(wT[:, :, kt, :], wf[:, kt, :])

    # ---- per-group routing state ----
    ct_tiles = [
        [sm.tile([P, JT, GS, 32], F16, name="ct") for _ in range(2)] for _ in range(NG)
    ]
    for g in range(NG):
        for tt in ct_tiles[g]:
            nc.gpsimd.memset(tt[:], 0.0)
        nc.gpsimd.memset(ct_tiles[g][0][:, :, :, 0:NI], 1.0 / NI)

    rinv_g = [None] * NG
    masked_g = [None] * NG
    maskedT_g = [None] * NG

    def emit_A(g, t):
        """cx = C x  (strip-packed, contract j), evacuate, DMA-transpose."""
        ct = ct_tiles[g][t % 2]
        pcx = psum.tile([P, ND], F32, name="pcx", tag="pcx", bufs=2)
        for jt in range(JT):
            for a in range(GS):
                nc.tensor.matmul(
                    pcx[ts(a, 32), 0:NK],
                    lhsT=ct[:, jt, a, :],
                    rhs=xn_g[g][:, a, jt, :],
                    start=(jt == 0),
                    stop=(jt == JT - 1),
                    tile_position=(0, 32 * a),
                    skip_group_check=True,
                )
        cx = rt.tile([P, NK], F16, name="cx")
        nc.vector.tensor_copy(cx[:], pcx[:, 0:NK])
        cxT = rt.tile([P, KT, P], F16, name="cxT")
        nc.sync.dma_start_transpose(cxT[:], cx[:])
        return cxT

    def emit_C(g, t, cxT):
        """s = cx W; mask to block-diag; norms; rinv; maskedT."""
        ps_s = psum.tile([P, ND], F32, name="ps_s", tag="ps", bufs=2)
        for kt in range(KT):
            nc.tensor.matmul(
                ps_s[:],
                lhsT=cxT[:, kt, :],
                rhs=wf[:, kt, :],
                start=(kt == 0),
                stop=(kt == KT - 1),
            )
        masked = rt.tile([P, ND], F16, name="masked")
        nc.vector.tensor_tensor(masked[:], ps_s[:], mask[:], op=ALU.mult)
        maskedT = rt.tile([P, IKT, P], F16, name="maskedT")
        nc.sync.dma_start_transpose(maskedT[:], masked[:])
        sq = rt.tile([P, ND], F16, name="sq")
        n2 = rt.tile([P, 1], F32, name="n2")
        nc.scalar.activation(sq[:], masked[:], AF.Square, accum_out=n2[:])
        # rinv = (n2+eps)^-0.5 on DVE: magic-constant guess + 2 Newton steps
        xe = rt.tile([P, 1], F32, name="xe")
        nc.vector.tensor_scalar(xe[:], n2[:], EPS, None, op0=ALU.add)
        xh = rt.tile([P, 1], F32, name="xh")
        nc.vector.tensor_scalar(xh[:], xe[:], 0.5, None, op0=ALU.mult)
        yt = rt.tile([P, 1], F32, name="yt")
        nc.vector.tensor_scalar(
            yt.bitcast(I32)[:], xe.bitcast(I32)[:], 1, None,
            op0=ALU.logical_shift_right,
        )
        nc.vector.tensor_scalar(
            yt.bitcast(I32)[:], yt.bitcast(I32)[:], 0x5F3759E0, None,
            op0=ALU.subtract,
        )
        nc.vector.tensor_scalar(
            yt.bitcast(I32)[:], yt.bitcast(I32)[:], -1, None,
            op0=ALU.bitwise_xor,
        )
        y2 = rt.tile([P, 1], F32, name="y2")
        for _ in range(2):
            nc.vector.tensor_tensor(y2[:], yt[:], yt[:], op=ALU.mult)
            nc.vector.tensor_tensor(y2[:], y2[:], xh[:], op=ALU.mult)
            nc.vector.tensor_scalar(y2[:], y2[:], -1.0, 1.5, op0=ALU.mult, op1=ALU.add)
            nc.vector.tensor_tensor(yt[:], yt[:], y2[:], op=ALU.mult)
        rinv_g[g] = yt
        masked_g[g] = masked
        maskedT_g[g] = maskedT

    def emit_wv_b(g, t):
        """w_v = W^T-blocks . v (tiny), then b = w_v . xT (strip-packed)."""
        maskedT = maskedT_g[g]
        ps_wv = psum.tile([P, ND], F32, name="ps_wv", tag="paux", bufs=1)
        for kt in range(KT):
            for ikt in range(IKT):
                nc.tensor.matmul(
                    ps_wv[:, ts(kt, P)],
                    lhsT=wT[:, ikt, kt, :],
                    rhs=maskedT[:, ikt, :],
                    start=(ikt == 0),
                    stop=(ikt == IKT - 1),
                    skip_group_check=True,
                )
        wv = rt.tile([P, KT, P], F16, name="wv")
        nc.scalar.copy(wv.rearrange("p t q -> p (t q)"), ps_wv[:, 0 : KT * P])
        bsc = rt.tile([P, 2, ND], F16, name="bsc")
        for jc in range(2):
            ps_b = psum.tile([P, ND], F32, name="ps_b", tag="pb", bufs=3)
            for kt in range(KT):
                for a in range(GS):
                    nc.tensor.matmul(
                        ps_b[ts(a, 32), :],
                        lhsT=wv[:, kt, ts(a, 32)],
                        rhs=xt_g[g][:, a, kt, ts(jc, ND)],
                        start=(kt == 0),
                        stop=(kt == KT - 1),
                        tile_position=(0, 32 * a),
                        skip_group_check=True,
                    )
            nc.scalar.activation(bsc[:, jc, :], ps_b[:], AF.Exp, scale=rinv_g[g][:])
        return bsc

    def emit_soft(g, t, bsc):
        """bT via DMA transpose; softmax over i; write c_{t+1}."""
        bT = rt.tile([P, JT, P], F16, name="bT")
        nc.sync.dma_start_transpose(bT[:], bsc.rearrange("p c d -> p (c d)"))
        expT = bT.rearrange("p t (s c) -> p t s c", c=32)[:, :, :, 0:NI]
        zsum = rt.tile([P, JT, GS], F32, name="zsum")
        nc.vector.tensor_reduce(zsum[:], expT, axis=AX.X, op=ALU.add)
        rz = rt.tile([P, JT, GS], F32, name="rz")
        nc.vector.reciprocal(rz[:], zsum[:])
        ct_next = ct_tiles[g][(t + 1) % 2]
        nc.vector.tensor_tensor(
            ct_next[:, :, :, 0:NI],
            expT,
            rz.unsqueeze(3).broadcast_to([P, JT, GS, NI]),
            op=ALU.mult,
        )

    def emit_final(g):
        """diag-extract via sel matmul from maskedT, scale by rinv, DMA out."""
        maskedT = maskedT_g[g]
        ps_v = psum.tile([P, ND], F32, name="ps_v", tag="paux", bufs=1)
        for kt in range(IKT):
            nc.tensor.matmul(
                ps_v[:, 0:DK],
                lhsT=maskedT[:, kt, :],
                rhs=sel[:],
                start=(kt == 0),
                stop=(kt == IKT - 1),
            )
        vout = rt.tile([P, DK], F32, name="vout")
        nc.scalar.activation(vout[:], ps_v[:, 0:DK], AF.Copy, scale=rinv_g[g][:])
        for a in range(GS):
            nc.gpsimd.dma_start(
                out_ap[g * GS + a], vout[32 * a : 32 * a + NI, :]
            )

    for t in range(ROUTINGS):
        cxTs = [emit_A(g, t) for g in range(NG)]
        for g in range(NG):
            emit_C(g, t, cxTs[g])
        if t < ROUTINGS - 1:
            bscs = [emit_wv_b(g, t) for g in range(NG)]
            for g in range(NG):
                emit_soft(g, t, bscs[g])
        else:
            for g in range(NG):
                emit_final(g)


def _np_consts():
    mask = np.zeros((P, ND), dtype=np.float16)
    for a in range(GS):
        for i in range(NI):
            mask[32 * a + i, DK * i : DK * (i + 1)] = 1.0
    sel = np.tile(np.eye(DK, dtype=np.float16), (IKT, 1))
    return mask, sel


@functools.cache
def _build_nc():
    from contextlib import ExitStack

    nc = bacc.Bacc(
        "TRN2",
        target_bir_lowering=False,
        debug=False,
        num_devices=NCORES,
    )
    x_t = nc.dram_tensor("x", [BSH, NJ, NK], F16, kind="ExternalInput")
    xt_t = nc.dram_tensor("xt", [BSH, NK, NJ], F16, kind="ExternalInput")
    w_t = nc.dram_tensor("w", [NK, ND], F32, kind="ExternalInput")
    mask_t = nc.dram_tensor("mask", [P, ND], F16, kind="ExternalInput")
    sel_t = nc.dram_tensor("sel", [P, DK], F16, kind="ExternalInput")
    out_t = nc.dram_tensor("out", [BSH, NI, DK], F32, kind="ExternalOutput")

    with tile.TileContext(nc) as tc:
        with ExitStack() as ctx:
            _build_body(
                nc,
                tc,
                x_t.ap(),
                xt_t.ap(),
                w_t.ap(),
                mask_t.ap(),
                sel_t.ap(),
                out_t.ap(),
                ctx,
            )
    nc.compile()
    return nc


def _in_maps(x, W):
    x = np.asarray(x, dtype=np.float32)
    w2d = np.ascontiguousarray(np.asarray(W, dtype=np.float32).reshape(NK, ND))
    mask, sel = _np_consts()
    x16 = x.astype(np.float16)
    maps = []
    for c in range(NCORES):
        xs = np.ascontiguousarray(x16[c * BSH : (c + 1) * BSH])
        xts = np.ascontiguousarray(xs.transpose(0, 2, 1))
        maps.append({"x": xs, "xt": xts, "w": w2d, "mask": mask, "sel": sel})
    return maps


def run(x, W, trace=False):
    nc = _build_nc()
    res = run_bass_kernel_spmd(nc, _in_maps(x, W), list(range(NCORES)), trace=trace)
    out = np.concatenate([r["out"] for r in res.results], axis=0)
    return out.astype(np.float32), res


def kernel(x, W):
    out, _ = run(x, W, trace=False)
    return out
